# revision 3
# baseline (speedup 1.0000x reference)
"""Trainium2 Bass kernel for nn_Adapter (LayerNorm -> down-proj -> GELU ->
up-proj -> residual), data-parallel over 8 NeuronCores (one batch row each).

Per-core pipeline (x: [4096, 2048] fp32):
  1. DMA a 256-token group of x.
  2. LayerNorm stats in fp32 on DVE (bn_stats/bn_aggr), rstd via
     reciprocal-seeded Newton iteration (avoids ACT table switches away
     from the gelu set).
  3. Normalize + cast to bf16 in one ScalarE pass (per-partition scale/bias).
  4. Transpose xn via TensorE (is_transpose matmuls, 8 chunks packed per
     PSUM bank), ScalarE copies PSUM->SBUF.
  5. mm1: S^T[a,t] accumulated over 16 K-chunks, W1 chunk stationary.
  6. GELU on ScalarE straight out of PSUM (bf16 out = h^T, already in
     the layout mm2 needs).
  7. mm2: out[t,d] with h^T token-slices stationary, W2 moving.
  8. Residual add (fp32, PSUM + x) on DVE, DMA out.

All weight prep is done host-side in numpy: gamma folded into W1,
beta/b_down folded into a GELU pre-bias v, bf16 casts, identity matrix.
"""

import os
from contextlib import ExitStack

import numpy as np

import concourse.bass as bass
import concourse.tile as tile
from concourse import mybir
from concourse.bass_utils import run_bass_kernel_spmd

T, D, A = 4096, 2048, 256
NCORES = 8
P = 128
GSUB = 2  # 128-token subtiles per group
GT = P * GSUB  # tokens per group
NGRP = T // GT
NCHUNK = D // P  # K-chunks for mm1
EPS = 1e-5

F32 = mybir.dt.float32
BF16 = mybir.dt.bfloat16
AF = mybir.ActivationFunctionType
OP = mybir.AluOpType


def _split_sync_waits(nc, max_waits=1):
    """walrus in this env rejects >1 sync-wait on ctrl instructions; move
    excess waits onto NoOps inserted before the instruction (same engine)."""
    idx = 0
    for f in nc.m.functions:
        for bb in f.blocks:
            new_insts = []
            for inst in bb.instructions:
                si = inst.sync_info
                waits = list(si.on_wait) if si is not None and si.on_wait else []
                if len(waits) > max_waits:
                    while len(waits) > max_waits:
                        chunk, waits = waits[:1], waits[1:]
                        nop = mybir.InstNoOp(name=f"waitsplit_{idx}", ins=[], outs=[])
                        idx += 1
                        nop.engine = inst.engine
                        nop.sync_info = mybir.SyncInfo(on_wait=chunk, on_update=[])
                        new_insts.append(nop)
                    si.on_wait = waits
                new_insts.append(inst)
            bb.instructions[:] = new_insts
    return idx


def build_nc(v_nonzero: bool, bup_nonzero: bool):
    nc = bass.Bass()
    x_ext = nc.declare_dram_parameter("x", [T, D], F32, isOutput=False)
    w1_ext = nc.declare_dram_parameter("w1", [D, A], BF16, isOutput=False)
    w2_ext = nc.declare_dram_parameter("w2", [A, D], BF16, isOutput=False)
    id_ext = nc.declare_dram_parameter("ident", [P, P], BF16, isOutput=False)
    v_ext = (
        nc.declare_dram_parameter("v", [A], F32, isOutput=False) if v_nonzero else None
    )
    bup_ext = (
        nc.declare_dram_parameter("bup", [D], F32, isOutput=False)
        if bup_nonzero
        else None
    )
    out_ext = nc.declare_dram_parameter("out", [T, D], F32, isOutput=True)

    with tile.TileContext(nc) as tc, ExitStack() as ctx:
        const = ctx.enter_context(tc.tile_pool(name="const", bufs=1))
        w1_t = const.tile([P, NCHUNK, A], BF16)  # [d_in, chunk, a]
        nc.sync.dma_start(out=w1_t, in_=w1_ext.rearrange("(c p) a -> p c a", p=P))
        w2_t = const.tile([P, 2, D], BF16)  # [a_in, a_chunk, d]
        nc.sync.dma_start(out=w2_t, in_=w2_ext.rearrange("(c p) d -> p c d", p=P))
        ident = const.tile([P, P], BF16)
        nc.sync.dma_start(out=ident, in_=id_ext[:, :])
        if v_ext is not None:
            v_t = const.tile([P, 2], F32)
            nc.sync.dma_start(out=v_t, in_=v_ext.rearrange("(c p) -> p c", p=P))
        if bup_ext is not None:
            bup_t = const.tile([P, D], F32)
            bup_bcast = bass.AP(
                tensor=bup_ext.tensor,
                offset=bup_ext.offset,
                ap=[[0, P], bup_ext.ap[0]],
            )
            nc.gpsimd.dma_start(out=bup_t, in_=bup_bcast)

        xpool = ctx.enter_context(tc.tile_pool(name="x", bufs=3))
        xnpool = ctx.enter_context(tc.tile_pool(name="xn", bufs=3))
        xntpool = ctx.enter_context(tc.tile_pool(name="xnt", bufs=2))
        htpool = ctx.enter_context(tc.tile_pool(name="ht", bufs=2))
        outpool = ctx.enter_context(tc.tile_pool(name="o", bufs=2))
        sm = ctx.enter_context(tc.tile_pool(name="sm", bufs=4))
        tp_ps = ctx.enter_context(tc.tile_pool(name="tp_ps", bufs=2, space="PSUM"))
        mm1_ps = ctx.enter_context(tc.tile_pool(name="mm1_ps", bufs=2, space="PSUM"))
        mm2_ps = ctx.enter_context(tc.tile_pool(name="mm2_ps", bufs=2, space="PSUM"))

        for g in range(NGRP):
            t0 = g * GT
            x_g = xpool.tile([P, GSUB, D], F32, tag="x")
            nc.sync.dma_start(
                out=x_g, in_=x_ext[t0 : t0 + GT, :].rearrange("(s p) d -> p s d", p=P)
            )

            # ---- stats (fp32) ----
            mv2 = sm.tile([P, GSUB, 2], F32, tag="mv2")
            for sl in range(GSUB):
                stats = sm.tile([P, 4, 6], F32, tag="stats")
                for ch in range(4):
                    nc.vector.bn_stats(
                        out=stats[:, ch, :], in_=x_g[:, sl, ch * 512 : (ch + 1) * 512]
                    )
                nc.vector.bn_aggr(out=mv2[:, sl, :], in_=stats)

            # ---- rstd = rsqrt(var+eps): reciprocal seed + 1 Newton step ----
            tvar = sm.tile([P, GSUB], F32, tag="tvar")
            nc.vector.tensor_scalar_add(out=tvar, in0=mv2[:, :, 1], scalar1=EPS)
            inv = sm.tile([P, GSUB], F32, tag="inv")
            nc.vector.reciprocal(out=inv, in_=tvar)
            y0 = sm.tile([P, GSUB], F32, tag="y0")
            nc.vector.tensor_scalar(
                out=y0, in0=inv, scalar1=0.5, scalar2=0.5, op0=OP.mult, op1=OP.add
            )
            th = sm.tile([P, GSUB], F32, tag="th")
            nc.vector.tensor_scalar_mul(out=th, in0=tvar, scalar1=0.5)
            yy = sm.tile([P, GSUB], F32, tag="yy")
            nc.vector.tensor_mul(out=yy, in0=y0, in1=y0)
            wt = sm.tile([P, GSUB], F32, tag="wt")
            nc.vector.tensor_mul(out=wt, in0=yy, in1=th)
            w2c = sm.tile([P, GSUB], F32, tag="w2c")
            nc.vector.tensor_scalar(
                out=w2c, in0=wt, scalar1=-1.0, scalar2=1.5, op0=OP.mult, op1=OP.add
            )
            rstd = sm.tile([P, GSUB], F32, tag="rstd")
            nc.vector.tensor_mul(out=rstd, in0=y0, in1=w2c)
            # negmu = -(mean * rstd)
            negmu = sm.tile([P, GSUB], F32, tag="negmu")
            nc.vector.tensor_mul(out=negmu, in0=mv2[:, :, 0], in1=rstd)
            nc.vector.tensor_scalar_mul(out=negmu, in0=negmu, scalar1=-1.0)

            # ---- normalize + cast to bf16, then transpose ----
            xnT = xntpool.tile([P, NCHUNK, GT], BF16, tag="xnt")  # [d, chunk, t]
            for sl in range(GSUB):
                xn_t = xnpool.tile([P, D], BF16, tag="xn")
                nc.scalar.activation(
                    out=xn_t,
                    in_=x_g[:, sl, :],
                    func=AF.Identity,
                    bias=negmu[:, sl : sl + 1],
                    scale=rstd[:, sl : sl + 1],
                )
                for w in range(2):  # two waves of 8 chunk-transposes per bank
                    tp = tp_ps.tile([P, 8, P], BF16, tag="tp")
                    for cc in range(8):
                        c = w * 8 + cc
                        nc.tensor.transpose(
                            tp[:, cc, :], xn_t[:, c * P : (c + 1) * P], ident
                        )
                    nc.scalar.copy(
                        out=xnT[:, w * 8 : (w + 1) * 8, sl * P : (sl + 1) * P],
                        in_=tp,
                    )

            # ---- mm1: S^T[a, t] over 16 K-chunks (two a-halves) ----
            ht = htpool.tile([P, 2, GT], BF16, tag="ht")  # [a, a_chunk, t]
            for h in range(2):
                ps1 = mm1_ps.tile([P, GT], F32, tag=f"mm1_{h}")
                for c in range(NCHUNK):
                    nc.tensor.matmul(
                        ps1,
                        lhsT=w1_t[:, c, h * P : (h + 1) * P],
                        rhs=xnT[:, c, :],
                        start=(c == 0),
                        stop=(c == NCHUNK - 1),
                    )
                if v_ext is not None:
                    nc.scalar.activation(
                        out=ht[:, h, :],
                        in_=ps1,
                        func=AF.Gelu,
                        bias=v_t[:, h : h + 1],
                        scale=1.0,
                    )
                else:
                    nc.scalar.activation(out=ht[:, h, :], in_=ps1, func=AF.Gelu)

            # ---- mm2 + residual + store ----
            out_g = outpool.tile([P, GSUB, D], F32, tag="o")
            for sl in range(GSUB):
                for nh in range(4):
                    ps2 = mm2_ps.tile([P, 512], F32, tag="mm2")
                    for a2 in range(2):
                        nc.tensor.matmul(
                            ps2,
                            lhsT=ht[:, a2, sl * P : (sl + 1) * P],
                            rhs=w2_t[:, a2, nh * 512 : (nh + 1) * 512],
                            start=(a2 == 0),
                            stop=(a2 == 1),
                        )
                    seg = slice(nh * 512, (nh + 1) * 512)
                    nc.vector.tensor_add(
                        out=out_g[:, sl, seg], in0=ps2, in1=x_g[:, sl, seg]
                    )
                    if bup_ext is not None:
                        nc.vector.tensor_add(
                            out=out_g[:, sl, seg],
                            in0=out_g[:, sl, seg],
                            in1=bup_t[:, seg],
                        )
            nc.sync.dma_start(
                out=out_ext[t0 : t0 + GT, :].rearrange("(s p) d -> p s d", p=P),
                in_=out_g,
            )

    _split_sync_waits(nc)
    return nc


_CACHE = {}


def _get_nc(v_nonzero, bup_nonzero):
    key = (v_nonzero, bup_nonzero)
    if key not in _CACHE:
        _CACHE[key] = build_nc(v_nonzero, bup_nonzero)
    return _CACHE[key]


def kernel(
    hidden_states, ln_gamma, ln_beta, w_down, b_down, w_up, b_up
) -> np.ndarray:
    import ml_dtypes

    hidden_states = np.asarray(hidden_states, dtype=np.float32)
    ln_gamma = np.asarray(ln_gamma, dtype=np.float32)
    ln_beta = np.asarray(ln_beta, dtype=np.float32)
    w_down = np.asarray(w_down, dtype=np.float32)
    b_down = np.asarray(b_down, dtype=np.float32)
    w_up = np.asarray(w_up, dtype=np.float32)
    b_up = np.asarray(b_up, dtype=np.float32)

    w1 = (ln_gamma[:, None] * w_down).astype(ml_dtypes.bfloat16)
    w2 = w_up.astype(ml_dtypes.bfloat16)
    v = ln_beta @ w_down + b_down
    ident = np.eye(P, dtype=ml_dtypes.bfloat16)
    v_nonzero = bool(np.any(v != 0))
    bup_nonzero = bool(np.any(b_up != 0))

    nc = _get_nc(v_nonzero, bup_nonzero)

    in_maps = []
    for c in range(NCORES):
        m = {
            "x": np.ascontiguousarray(hidden_states[c]),
            "w1": w1,
            "w2": w2,
            "ident": ident,
        }
        if v_nonzero:
            m["v"] = v.astype(np.float32)
        if bup_nonzero:
            m["bup"] = b_up
        in_maps.append(m)

    trace = bool(int(os.environ.get("ADAPTER_KERNEL_TRACE", "0")))
    res = run_bass_kernel_spmd(
        nc, in_maps, core_ids=list(range(NCORES)), trace=trace
    )
    kernel.last_result = res
    out = np.stack([res.results[c]["out"] for c in range(NCORES)], axis=0)
    return out


# revision 6
# speedup vs baseline: 1.0874x; 1.0874x over previous
"""Trainium2 Bass kernel for nn_Adapter (LayerNorm -> down-proj -> GELU ->
up-proj -> residual), data-parallel over 8 NeuronCores (one batch row each).

Per-core pipeline (x: [4096, 2048] fp32):
  1. DMA a 256-token group of x.
  2. LayerNorm stats in fp32 on DVE (bn_stats/bn_aggr), rstd via
     reciprocal-seeded Newton iteration (avoids ACT table switches away
     from the gelu set).
  3. Normalize + cast to bf16 in one ScalarE pass (per-partition scale/bias).
  4. Transpose xn via TensorE (is_transpose matmuls, 8 chunks packed per
     PSUM bank), ScalarE copies PSUM->SBUF.
  5. mm1: S^T[a,t] accumulated over 16 K-chunks, W1 chunk stationary.
  6. GELU on ScalarE straight out of PSUM (bf16 out = h^T, already in
     the layout mm2 needs).
  7. mm2: out[t,d] with h^T token-slices stationary, W2 moving.
  8. Residual add (fp32, PSUM + x) on DVE, DMA out.

All weight prep is done host-side in numpy: gamma folded into W1,
beta/b_down folded into a GELU pre-bias v, bf16 casts, identity matrix.
"""

import os
from contextlib import ExitStack

import numpy as np

import concourse.bass as bass
import concourse.tile as tile
from concourse import mybir
from concourse.bass_utils import run_bass_kernel_spmd

T, D, A = 4096, 2048, 256
NCORES = 8
P = 128
GSUB = 2  # 128-token subtiles per group
GT = P * GSUB  # tokens per group
NGRP = T // GT
NCHUNK = D // P  # K-chunks for mm1
EPS = 1e-5

F32 = mybir.dt.float32
BF16 = mybir.dt.bfloat16
AF = mybir.ActivationFunctionType
OP = mybir.AluOpType


def _split_sync_waits(nc, max_waits=1):
    """walrus in this env rejects >1 sync-wait on ctrl instructions; move
    excess waits onto NoOps inserted before the instruction (same engine)."""
    idx = 0
    for f in nc.m.functions:
        for bb in f.blocks:
            new_insts = []
            for inst in bb.instructions:
                si = inst.sync_info
                waits = list(si.on_wait) if si is not None and si.on_wait else []
                if len(waits) > max_waits:
                    while len(waits) > max_waits:
                        chunk, waits = waits[:1], waits[1:]
                        nop = mybir.InstNoOp(name=f"waitsplit_{idx}", ins=[], outs=[])
                        idx += 1
                        nop.engine = inst.engine
                        nop.sync_info = mybir.SyncInfo(on_wait=chunk, on_update=[])
                        new_insts.append(nop)
                    si.on_wait = waits
                new_insts.append(inst)
            bb.instructions[:] = new_insts
    return idx


def build_nc(v_nonzero: bool, bup_nonzero: bool):
    nc = bass.Bass()
    x_ext = nc.declare_dram_parameter("x", [T, D], F32, isOutput=False)
    w1_ext = nc.declare_dram_parameter("w1", [D, A], BF16, isOutput=False)
    w2_ext = nc.declare_dram_parameter("w2", [A, D], BF16, isOutput=False)
    id_ext = nc.declare_dram_parameter("ident", [P, P], BF16, isOutput=False)
    v_ext = (
        nc.declare_dram_parameter("v", [A], F32, isOutput=False) if v_nonzero else None
    )
    bup_ext = (
        nc.declare_dram_parameter("bup", [D], F32, isOutput=False)
        if bup_nonzero
        else None
    )
    out_ext = nc.declare_dram_parameter("out", [T, D], F32, isOutput=True)

    with tile.TileContext(nc) as tc, ExitStack() as ctx:
        const = ctx.enter_context(tc.tile_pool(name="const", bufs=1))
        w1_t = const.tile([P, NCHUNK, A], BF16)  # [d_in, chunk, a]
        nc.sync.dma_start(out=w1_t, in_=w1_ext.rearrange("(c p) a -> p c a", p=P))
        w2_t = const.tile([P, 2, D], BF16)  # [a_in, a_chunk, d]
        nc.sync.dma_start(out=w2_t, in_=w2_ext.rearrange("(c p) d -> p c d", p=P))
        ident = const.tile([P, P], BF16)
        nc.sync.dma_start(out=ident, in_=id_ext[:, :])
        if v_ext is not None:
            v_t = const.tile([P, 2], F32)
            nc.sync.dma_start(out=v_t, in_=v_ext.rearrange("(c p) -> p c", p=P))
        if bup_ext is not None:
            bup_t = const.tile([P, D], F32)
            bup_bcast = bass.AP(
                tensor=bup_ext.tensor,
                offset=bup_ext.offset,
                ap=[[0, P], bup_ext.ap[0]],
            )
            nc.gpsimd.dma_start(out=bup_t, in_=bup_bcast)

        xpool = ctx.enter_context(tc.tile_pool(name="x", bufs=3))
        xnpool = ctx.enter_context(tc.tile_pool(name="xn", bufs=3))
        xntpool = ctx.enter_context(tc.tile_pool(name="xnt", bufs=2))
        htpool = ctx.enter_context(tc.tile_pool(name="ht", bufs=2))
        outpool = ctx.enter_context(tc.tile_pool(name="o", bufs=2))
        sm = ctx.enter_context(tc.tile_pool(name="sm", bufs=4))
        tp_ps = ctx.enter_context(tc.tile_pool(name="tp_ps", bufs=2, space="PSUM"))
        mm1_ps = ctx.enter_context(tc.tile_pool(name="mm1_ps", bufs=2, space="PSUM"))
        mm2_ps = ctx.enter_context(tc.tile_pool(name="mm2_ps", bufs=2, space="PSUM"))

        for g in range(NGRP):
            t0 = g * GT
            x_g = xpool.tile([P, GSUB, D], F32, tag="x")
            nc.sync.dma_start(
                out=x_g, in_=x_ext[t0 : t0 + GT, :].rearrange("(s p) d -> p s d", p=P)
            )

            # ---- stats (fp32 accumulators) ----
            # sumsq via ScalarE Square+accum, sum via DVE tensor_scalar+accum
            sumsq = sm.tile([P, GSUB], F32, tag="sumsq")
            sumx = sm.tile([P, GSUB], F32, tag="sumx")
            for sl in range(GSUB):
                sq_junk = xnpool.tile([P, D], BF16, tag="sqj")
                nc.scalar.activation(
                    out=sq_junk,
                    in_=x_g[:, sl, :],
                    func=AF.Square,
                    accum_out=sumsq[:, sl : sl + 1],
                )
                sx_junk = xnpool.tile([P, D], BF16, tag="sxj")
                nc.vector.tensor_scalar(
                    out=sx_junk,
                    in0=x_g[:, sl, :],
                    scalar1=1.0,
                    scalar2=0.0,
                    op0=OP.mult,
                    op1=OP.add,
                    accum_out=sumx[:, sl : sl + 1],
                )

            # ---- rstd = rsqrt(var+eps); mostly on GpSimd ----
            mu = sm.tile([P, GSUB], F32, tag="mu")
            nc.gpsimd.tensor_scalar_mul(out=mu, in0=sumx, scalar1=1.0 / D)
            t1 = sm.tile([P, GSUB], F32, tag="t1")
            nc.gpsimd.tensor_scalar(
                out=t1, in0=sumsq, scalar1=1.0 / D, scalar2=EPS, op0=OP.mult, op1=OP.add
            )
            musq = sm.tile([P, GSUB], F32, tag="musq")
            nc.gpsimd.tensor_mul(out=musq, in0=mu, in1=mu)
            tvar = sm.tile([P, GSUB], F32, tag="tvar")
            nc.gpsimd.tensor_tensor(out=tvar, in0=t1, in1=musq, op=OP.subtract)
            inv = sm.tile([P, GSUB], F32, tag="inv")
            nc.vector.reciprocal(out=inv, in_=tvar)
            y0 = sm.tile([P, GSUB], F32, tag="y0")
            nc.vector.tensor_scalar(
                out=y0, in0=inv, scalar1=0.5, scalar2=0.5, op0=OP.mult, op1=OP.add
            )
            th = sm.tile([P, GSUB], F32, tag="th")
            nc.gpsimd.tensor_scalar_mul(out=th, in0=tvar, scalar1=0.5)
            yy = sm.tile([P, GSUB], F32, tag="yy")
            nc.gpsimd.tensor_mul(out=yy, in0=y0, in1=y0)
            wt = sm.tile([P, GSUB], F32, tag="wt")
            nc.gpsimd.tensor_mul(out=wt, in0=yy, in1=th)
            w2c = sm.tile([P, GSUB], F32, tag="w2c")
            nc.gpsimd.tensor_scalar(
                out=w2c, in0=wt, scalar1=-1.0, scalar2=1.5, op0=OP.mult, op1=OP.add
            )
            rstd = sm.tile([P, GSUB], F32, tag="rstd")
            nc.gpsimd.tensor_mul(out=rstd, in0=y0, in1=w2c)
            # negmu = -(mean * rstd)
            negmu = sm.tile([P, GSUB], F32, tag="negmu")
            nc.gpsimd.tensor_mul(out=negmu, in0=mu, in1=rstd)
            nc.gpsimd.tensor_scalar_mul(out=negmu, in0=negmu, scalar1=-1.0)

            # ---- normalize + cast to bf16, then transpose ----
            xnT = xntpool.tile([P, NCHUNK, GT], BF16, tag="xnt")  # [d, chunk, t]
            for sl in range(GSUB):
                xn_t = xnpool.tile([P, D], BF16, tag="xn")
                if sl == 0:
                    nc.scalar.activation(
                        out=xn_t,
                        in_=x_g[:, sl, :],
                        func=AF.Identity,
                        bias=negmu[:, sl : sl + 1],
                        scale=rstd[:, sl : sl + 1],
                    )
                else:
                    nc.vector.tensor_scalar(
                        out=xn_t,
                        in0=x_g[:, sl, :],
                        scalar1=rstd[:, sl : sl + 1],
                        scalar2=negmu[:, sl : sl + 1],
                        op0=OP.mult,
                        op1=OP.add,
                    )
                for w in range(2):  # two waves of 8 chunk-transposes per bank
                    tp = tp_ps.tile([P, 8, P], BF16, tag="tp")
                    for cc in range(8):
                        c = w * 8 + cc
                        nc.tensor.transpose(
                            tp[:, cc, :], xn_t[:, c * P : (c + 1) * P], ident
                        )
                    dst = xnT[:, w * 8 : (w + 1) * 8, sl * P : (sl + 1) * P]
                    if sl == 0:
                        nc.scalar.copy(out=dst, in_=tp)
                    else:
                        nc.vector.tensor_copy(out=dst, in_=tp)

            # ---- mm1: S^T[a, t] over 16 K-chunks (two a-halves) ----
            ht = htpool.tile([P, 2, GT], BF16, tag="ht")  # [a, a_chunk, t]
            for h in range(2):
                ps1 = mm1_ps.tile([P, GT], F32, tag=f"mm1_{h}", bufs=1)
                for c in range(NCHUNK):
                    nc.tensor.matmul(
                        ps1,
                        lhsT=w1_t[:, c, h * P : (h + 1) * P],
                        rhs=xnT[:, c, :],
                        start=(c == 0),
                        stop=(c == NCHUNK - 1),
                    )
                if v_ext is not None:
                    nc.scalar.activation(
                        out=ht[:, h, :],
                        in_=ps1,
                        func=AF.Gelu,
                        bias=v_t[:, h : h + 1],
                        scale=1.0,
                    )
                else:
                    nc.scalar.activation(out=ht[:, h, :], in_=ps1, func=AF.Gelu)

            # ---- mm2 + residual + store ----
            out_g = outpool.tile([P, GSUB, D], F32, tag="o")
            for sl in range(GSUB):
                for nh in range(2):
                    ps2 = mm2_ps.tile([P, 1024], F32, tag="mm2")
                    for sub in range(2):
                        for a2 in range(2):
                            nc.tensor.matmul(
                                ps2[:, sub * 512 : (sub + 1) * 512],
                                lhsT=ht[:, a2, sl * P : (sl + 1) * P],
                                rhs=w2_t[
                                    :,
                                    a2,
                                    nh * 1024 + sub * 512 : nh * 1024 + (sub + 1) * 512,
                                ],
                                start=(a2 == 0),
                                stop=(a2 == 1),
                            )
                    seg = slice(nh * 1024, (nh + 1) * 1024)
                    nc.vector.tensor_add(
                        out=out_g[:, sl, seg], in0=ps2, in1=x_g[:, sl, seg]
                    )
                    if bup_ext is not None:
                        nc.vector.tensor_add(
                            out=out_g[:, sl, seg],
                            in0=out_g[:, sl, seg],
                            in1=bup_t[:, seg],
                        )
            nc.sync.dma_start(
                out=out_ext[t0 : t0 + GT, :].rearrange("(s p) d -> p s d", p=P),
                in_=out_g,
            )

    _split_sync_waits(nc)
    return nc


_CACHE = {}


def _get_nc(v_nonzero, bup_nonzero):
    key = (v_nonzero, bup_nonzero)
    if key not in _CACHE:
        _CACHE[key] = build_nc(v_nonzero, bup_nonzero)
    return _CACHE[key]


def kernel(
    hidden_states, ln_gamma, ln_beta, w_down, b_down, w_up, b_up
) -> np.ndarray:
    import ml_dtypes

    hidden_states = np.asarray(hidden_states, dtype=np.float32)
    ln_gamma = np.asarray(ln_gamma, dtype=np.float32)
    ln_beta = np.asarray(ln_beta, dtype=np.float32)
    w_down = np.asarray(w_down, dtype=np.float32)
    b_down = np.asarray(b_down, dtype=np.float32)
    w_up = np.asarray(w_up, dtype=np.float32)
    b_up = np.asarray(b_up, dtype=np.float32)

    w1 = (ln_gamma[:, None] * w_down).astype(ml_dtypes.bfloat16)
    w2 = w_up.astype(ml_dtypes.bfloat16)
    v = ln_beta @ w_down + b_down
    ident = np.eye(P, dtype=ml_dtypes.bfloat16)
    v_nonzero = bool(np.any(v != 0))
    bup_nonzero = bool(np.any(b_up != 0))

    nc = _get_nc(v_nonzero, bup_nonzero)

    in_maps = []
    for c in range(NCORES):
        m = {
            "x": np.ascontiguousarray(hidden_states[c]),
            "w1": w1,
            "w2": w2,
            "ident": ident,
        }
        if v_nonzero:
            m["v"] = v.astype(np.float32)
        if bup_nonzero:
            m["bup"] = b_up
        in_maps.append(m)

    trace = bool(int(os.environ.get("ADAPTER_KERNEL_TRACE", "0")))
    res = run_bass_kernel_spmd(
        nc, in_maps, core_ids=list(range(NCORES)), trace=trace
    )
    kernel.last_result = res
    out = np.stack([res.results[c]["out"] for c in range(NCORES)], axis=0)
    return out
